# revision 23
# baseline (speedup 1.0000x reference)
"""Trainium2 Bass kernel for a transformer decoder block.

Shapes (hardcoded): B=4, S=1024, D=1024, H=16 heads, DH=64, FFN F=4096.

Sharding: 8 cores = 4 batches x 2 sequence-halves.  Core (b, h) handles
query rows {64*(2t+h)+r : t in 0..7, r in 0..63} of batch b (interleaved
64-row blocks so the causal-attention work per core is identical -> one
uniform SPMD program).  Each core recomputes the (small) K/V projections
it needs, so no collectives are required.

On-chip layout is feature-major ("transposed"): activations live as
[feature, token] so every matmul contraction sits on the partition axis.
The host pre-transposes inputs/weights and re-transposes the output.

Scheduling: engines execute their instruction streams in order, so each
attention head-pair's softmax (ScalarE-bound) is emitted with "filler"
projection matmul groups woven between its k-chunks, keeping the PE busy
while exps drain.  The FFN + final layernorm run in column halves so the
LN2/LN3 tails overlap FFN matmuls, with the ff2 weights held resident in
slots freed by the attention phase.
"""

import sys

if "/opt/trn_rl_repo" not in sys.path:
    sys.path.insert(0, "/opt/trn_rl_repo")

import numpy as np
import ml_dtypes

B, S, D, H, F, DH = 4, 1024, 1024, 16, 4096, 64
NCORES = 8
SQ = 512            # query rows per core
HQ = 256            # half of SQ (FFN column split)
NDT = D // 128      # 8 d-tiles
NFT = F // 128      # 32 f-tiles
NHP = H // 2        # 8 head pairs
NKC = S // 128      # 8 k chunks
BF16 = ml_dtypes.bfloat16

_PROG = None


def _build_program():
    import concourse.mybir as mybir
    from concourse import bacc
    from concourse.tile import TileContext

    f32 = mybir.dt.float32
    bf16 = mybir.dt.bfloat16
    f32r = mybir.dt.float32r
    AF = mybir.ActivationFunctionType
    OP = mybir.AluOpType

    nc = bacc.Bacc("TRN2", target_bir_lowering=False, debug=False,
                   num_devices=NCORES)

    def din(name, shape, dt=bf16):
        return nc.dram_tensor(name, shape, dt, kind="ExternalInput")

    # activations, partition-major so each loads with few contiguous DMAs
    xt_full = din("xt_full", [128, NDT, S])          # X^T (K/V source)
    xq = din("xq", [128, NDT, SQ])                   # X^T own q rows
    enc_t = din("enc_t", [128, NDT, S])              # encoder^T
    sa_mask = din("sa_mask", [128, NKC, 64])         # causal boundary slabs

    # weights staged host-side in exactly the sbuf tile layout
    w_sa_q = din("w_sa_q", [NHP, 128, NDT, 128])
    w_sa_k = din("w_sa_k", [NHP, 128, NDT, 128])
    w_sa_v = din("w_sa_v", [2, 128, NDT, 512])
    w_sa_o = din("w_sa_o", [NDT, 128, NDT, 128])
    w_ca_q = din("w_ca_q", [NHP, 128, NDT, 128])
    w_ca_k = din("w_ca_k", [NHP, 128, NDT, 128])
    w_ca_v = din("w_ca_v", [2, 128, NDT, 512])
    w_ca_o = din("w_ca_o", [NDT, 128, NDT, 128])
    w_ff1 = din("w_ff1", [NFT, 128, NDT, 128])
    w_ff2 = din("w_ff2", [NDT, 128, NFT, 128])
    # ff2 dt-pairs pre-packed [128, 2, NFT, 128] for the resident slabs
    w_ff2p = din("w_ff2p", [2, 128, 2, NFT, 128])

    # all small per-feature vectors concatenated: one DMA
    # cols: bq1 0:8 | bq2 8:16 | bo1 16:24 | bo2 24:32 | b2 32:40 |
    #       ln1g 40:48 | ln1b 48:56 | ln2g .. | ln3b 72:88 | b1 88:120
    NV = 120
    v_all = din("v_all", [128, NV], f32)

    out_t = nc.dram_tensor("out_t", [NDT, 128, SQ], f32, kind="ExternalOutput")

    with TileContext(nc) as tc:
        with tc.tile_pool(name="p_acc", bufs=2, space="PSUM") as p_acc, \
             tc.tile_pool(name="p_s", bufs=2, space="PSUM") as p_s, \
             tc.tile_pool(name="p_pav", bufs=1, space="PSUM") as p_pav, \
             tc.tile_pool(name="const", bufs=1) as cpool, \
             tc.tile_pool(name="big", bufs=1) as big, \
             tc.tile_pool(name="wcol", bufs=8) as wcol, \
             tc.tile_pool(name="wbig", bufs=3) as wbig, \
             tc.tile_pool(name="pt", bufs=2) as ptp, \
             tc.tile_pool(name="bc", bufs=4) as bcp, \
             tc.tile_pool(name="sm", bufs=1) as smp, \
             tc.tile_pool(name="tmp", bufs=2) as tmpp, \
             tc.tile_pool(name="outp", bufs=2) as outp:

            # ---------------- activation loads first (startup latency) ----
            # first XQ chunk and the first q weight lead the DMA queues so
            # the first matmul chain can start as early as possible
            XQ = big.tile([128, NDT, SQ], bf16, tag="outb")
            nc.sync.dma_start(out=XQ[:, 0:2, :], in_=xq[:, 0:2, :])
            wq_pre = []
            for hp in range(NHP):
                t = wcol.tile([128, NDT, 128], bf16, tag="wcol", name="wqt")
                nc.sync.dma_start(out=t[:], in_=w_sa_q[hp])
                wq_pre.append(t)
            for c in range(1, 4):
                nc.sync.dma_start(out=XQ[:, 2 * c:2 * c + 2, :],
                                  in_=xq[:, 2 * c:2 * c + 2, :])

            # ---------------- constants / small vectors ----------------
            # LN stat matmuls use 1/D so psum rows are mean / E[x^2] directly
            oned_f = cpool.tile([128, 1], f32)
            nc.vector.memset(oned_f[:], 1.0 / D)
            ones_r = cpool.tile([128, 1], f32r)
            nc.vector.tensor_copy(ones_r[:], oned_f[:])
            ones_pe = cpool.tile([1, 128], f32)
            nc.vector.memset(ones_pe[:], 1.0)
            ones_col = cpool.tile([128, 128], bf16)
            nc.vector.memset(ones_col[:], 1.0)
            eps_t = cpool.tile([1, 1], f32)
            nc.vector.memset(eps_t[:], 1e-12)

            VA = cpool.tile([128, NV], f32)
            nc.sync.dma_start(out=VA[:], in_=v_all[:])
            bq1_sb, bq2_sb = VA[:, 0:8], VA[:, 8:16]
            bo1_sb, bo2_sb = VA[:, 16:24], VA[:, 24:32]
            b2_sb = VA[:, 32:40]
            ln_sb = {j: (VA[:, 40 + 16 * (j - 1):48 + 16 * (j - 1)],
                         VA[:, 48 + 16 * (j - 1):56 + 16 * (j - 1)])
                     for j in (1, 2, 3)}
            b1_sb = VA[:, 88:120]

            MS = cpool.tile([128, NKC, 64], bf16)
            nc.sync.dma_start(out=MS[:], in_=sa_mask[:])

            XT = big.tile([128, NDT, S], bf16, tag="xt")

            # ---------------- filler-step builders ----------------
            # Each returned closure emits one psum matmul group; they are
            # woven between attention k-chunks to keep the PE fed while the
            # ScalarE runs the softmax exps.
            def q_steps(hp, wq_d, src_q, bq_sb, QT, halves=False,
                        pre=None):
                cell = {"w": pre}

                def run_h(cs):
                    def run():
                        if cell["w"] is None:
                            cell["w"] = wcol.tile([128, NDT, 128], bf16,
                                                  tag="wcol", name="wqt")
                            nc.sync.dma_start(out=cell["w"][:], in_=wq_d[hp])
                        wqt = cell["w"]
                        W = cs.stop - cs.start
                        pq = p_acc.tile([128, SQ], f32, tag="acc")
                        for dt in range(NDT):
                            nc.tensor.matmul(pq[:, 0:W], wqt[:, dt, :],
                                             src_q[:, dt, cs],
                                             start=(dt == 0),
                                             stop=(dt == NDT - 1))
                        nc.vector.tensor_scalar_add(QT[:, hp, cs], pq[:, 0:W],
                                                    bq_sb[:, hp:hp + 1])
                    return run
                if halves:
                    return [run_h(slice(0, HQ)), run_h(slice(HQ, SQ))]
                return [run_h(slice(0, SQ))]

            def k_steps(hp, wk_d, src_kv, KT):
                cell = {}

                def run_kh(kh):
                    def run():
                        if kh == 0:
                            cell["w"] = wcol.tile([128, NDT, 128], bf16,
                                                  tag="wcol", name="wkt")
                            nc.sync.dma_start(out=cell["w"][:], in_=wk_d[hp])
                        wkt = cell["w"]
                        pk = p_acc.tile([128, 512], f32, tag="acc")
                        for dt in range(NDT):
                            nc.tensor.matmul(
                                pk[:], wkt[:, dt, :],
                                src_kv[:, dt, 512 * kh:512 * (kh + 1)],
                                start=(dt == 0), stop=(dt == NDT - 1))
                        nc.vector.tensor_copy(
                            KT[:, hp, 512 * kh:512 * (kh + 1)], pk[:])
                    return run
                return [run_kh(0), run_kh(1)]

            def v_steps(g, wv_d, src_kv, V3A, V3B):
                cell = {}

                def run_kc(kc):
                    def run():
                        if kc == 0:
                            cell["w"] = wbig.tile([128, NDT, 512], bf16,
                                                  tag="wbig", name="wvt")
                            nc.sync.dma_start(out=cell["w"][:], in_=wv_d[g])
                        wvt = cell["w"]
                        pv = p_acc.tile([128, 4, 128], f32, tag="acc")
                        for dt in range(NDT):
                            nc.tensor.matmul(
                                pv[:, :, :],
                                src_kv[:, dt, 128 * kc:128 * (kc + 1)],
                                wvt[:, dt, :],
                                start=(dt == 0), stop=(dt == NDT - 1))
                        nc.vector.tensor_copy(
                            V3A[:, kc, 4 * g:4 * g + 4, 0:64], pv[:, :, 0:64])
                        nc.vector.tensor_copy(
                            V3B[:, kc, 4 * g:4 * g + 4, 64:128],
                            pv[:, :, 64:128])
                    return run
                return [run_kc(kc) for kc in range(NKC)]

            def attention(hp, QT, KT, V3A, V3B, ATTN, causal, fillers=(),
                          pe_bcast=False):
                # V3A head slab = [V_A(64) | ones] -> AV rows 0:64, denom row
                # 64.  V3B = [ones | pad(63) | V_B(64)] -> denom row 0, AV
                # rows 64:128.  The ones column makes the AV matmul emit the
                # softmax denominator for free (no separate 1-row matmuls).
                # Both heads' scores share ONE 2-bank psum tile so each chunk
                # needs a single (strided) exp activation.
                pavA = p_pav.tile([128, SQ], f32, tag="pavA")
                pavB = p_pav.tile([128, SQ], f32, tag="pavB")
                fillers = list(fillers)
                fi = 0
                for j in range(NKC):
                    n0 = 64 * j if causal else 0
                    s_ = p_s.tile([128, 2, SQ], f32, tag="s")
                    ks = slice(128 * j, 128 * (j + 1))
                    nc.tensor.matmul(s_[:, 0, n0:SQ], KT[0:64, hp, ks],
                                     QT[0:64, hp, n0:SQ], start=True, stop=True)
                    nc.tensor.matmul(s_[:, 1, n0:SQ], KT[64:128, hp, ks],
                                     QT[64:128, hp, n0:SQ], start=True,
                                     stop=True)
                    pt = ptp.tile([128, 2, SQ], bf16, tag="pt")
                    nc.scalar.activation(out=pt[:, :, n0:SQ],
                                         in_=s_[:, :, n0:SQ],
                                         func=AF.Exp, scale=0.125)
                    if causal:
                        nc.vector.tensor_mul(pt[:, 0, n0:n0 + 64],
                                             pt[:, 0, n0:n0 + 64], MS[:, j, :])
                        nc.vector.tensor_mul(pt[:, 1, n0:n0 + 64],
                                             pt[:, 1, n0:n0 + 64], MS[:, j, :])
                    # fillers go HERE (between scores and AV) so the PE chews
                    # on them while ScalarE exps this chunk
                    while fi < len(fillers) and fi * NKC < (j + 1) * len(fillers):
                        fillers[fi]()
                        fi += 1
                    st, sp = (j == 0), (j == NKC - 1)
                    nc.tensor.matmul(pavA[0:65, n0:SQ],
                                     V3A[:, j, hp, 0:65],
                                     pt[:, 0, n0:SQ], start=st, stop=sp)
                    nc.tensor.matmul(pavB[:, n0:SQ],
                                     V3B[:, j, hp, :],
                                     pt[:, 1, n0:SQ], start=st, stop=sp)
                while fi < len(fillers):
                    fillers[fi]()
                    fi += 1
                if pe_bcast:
                    # Final attention of a phase: the next pav user (the
                    # projection's stat tiles) has slack, so skip the full
                    # evacuation.  Copy just the two denominator rows to
                    # SBUF, broadcast them raw with 1-contraction matmuls
                    # (the PE is idle here), invert the broadcast, and
                    # normalize straight out of PSUM (one psum operand).
                    dn = bcp.tile([128, SQ], bf16, tag="dn")
                    nc.scalar.activation(out=dn[64:65, :], in_=pavA[64:65, :],
                                         func=AF.Identity, scale=1.0)
                    nc.scalar.activation(out=dn[0:1, :], in_=pavB[0:1, :],
                                         func=AF.Identity, scale=1.0)
                    pd = p_s.tile([128, 2, SQ], f32, tag="s")
                    nc.tensor.matmul(pd[:, 0, :], ones_col[64:65, :],
                                     dn[64:65, :], start=True, stop=True)
                    nc.tensor.matmul(pd[:, 1, :], ones_col[0:1, :],
                                     dn[0:1, :], start=True, stop=True)
                    rA = bcp.tile([128, SQ], f32, tag="bc")
                    rB = bcp.tile([128, SQ], f32, tag="bc")
                    nc.vector.reciprocal_approx_fast(out=rA[:], in_=pd[:, 0, :])
                    nc.vector.reciprocal_approx_fast(out=rB[:], in_=pd[:, 1, :])
                    nc.vector.tensor_mul(ATTN[0:64, hp, :], pavA[0:64, :],
                                         rA[0:64, :])
                    nc.vector.tensor_mul(ATTN[64:128, hp, :], pavB[64:128, :],
                                         rB[64:128, :])
                    return
                # Evacuate the accumulators to SBUF immediately so the pav
                # psum banks free up for the next attention (p_pav bufs=1).
                # partition_broadcast reads only partition 0; tensor_copy can
                # shift partitions sbuf->sbuf, so: copy out, shift the denom
                # rows to partition 0, recip, broadcast, normalize.
                cpA = bcp.tile([128, SQ], f32, tag="bc")
                cpB = bcp.tile([128, SQ], f32, tag="bc")
                nc.scalar.activation(out=cpA[0:65, :], in_=pavA[0:65, :],
                                     func=AF.Identity, scale=1.0)
                nc.scalar.activation(out=cpB[:, :], in_=pavB[:, :],
                                     func=AF.Identity, scale=1.0)
                ra = smp.tile([1, SQ], f32, tag="ra")
                rb = smp.tile([1, SQ], f32, tag="rb")
                nc.vector.tensor_copy(ra[:], cpA[64:65, :])
                nc.vector.reciprocal_approx_fast(out=ra[:], in_=ra[:])
                nc.vector.tensor_copy(rb[:], cpB[0:1, :])
                nc.vector.reciprocal_approx_fast(out=rb[:], in_=rb[:])
                DAt = bcp.tile([128, SQ], f32, tag="bc")
                DBt = bcp.tile([128, SQ], f32, tag="bc")
                nc.gpsimd.partition_broadcast(DAt[:], ra[:])
                nc.gpsimd.partition_broadcast(DBt[:], rb[:])
                nc.vector.tensor_mul(ATTN[0:64, hp, :], cpA[0:64, :],
                                     DAt[0:64, :])
                nc.vector.tensor_mul(ATTN[64:128, hp, :], cpB[64:128, :],
                                     DBt[64:128, :])

            def ln_stats(pst1, pst2, W, base, pe_bcast=False):
                """Turn accumulated sum / sum-sq psum rows (local cols 0:W,
                representing global cols base:base+W) into broadcast mean +
                rstd.  Returns (mk, in_psum): mk(global col slice) ->
                (mean_ap, rstd_ap)."""
                m1 = smp.tile([1, SQ], f32, tag="m1")
                nc.vector.tensor_copy(m1[:, 0:W], pst1[0:1, 0:W])
                sq1 = smp.tile([1, SQ], f32, tag="ra")
                nc.scalar.activation(out=sq1[:, 0:W], in_=pst1[0:1, 0:W],
                                     func=AF.Square)
                varp = smp.tile([1, SQ], f32, tag="varp")
                nc.vector.tensor_sub(varp[:, 0:W], pst2[0:1, 0:W],
                                     sq1[:, 0:W])
                sv = smp.tile([1, SQ], f32, tag="rb")
                nc.scalar.activation(out=sv[:, 0:W], in_=varp[:, 0:W],
                                     func=AF.Sqrt, bias=eps_t[:],
                                     scale=float(D) / (D - 1))
                rstd = smp.tile([1, SQ], f32, tag="rstd")
                nc.vector.reciprocal_approx_fast(out=rstd[:, 0:W],
                                                 in_=sv[:, 0:W])
                if pe_bcast:
                    # PE is otherwise idle (end of kernel): broadcast via a
                    # 1-contraction matmul instead of the slower gpsimd path.
                    pd = p_s.tile([128, 2, SQ], f32, tag="s")
                    nc.tensor.matmul(pd[:, 0, 0:W], ones_pe[:], m1[:, 0:W],
                                     start=True, stop=True)
                    nc.tensor.matmul(pd[:, 1, 0:W], ones_pe[:], rstd[:, 0:W],
                                     start=True, stop=True)

                    def mk(c):
                        return (pd[:, 0, c.start - base:c.stop - base],
                                pd[:, 1, c.start - base:c.stop - base])
                    return mk, True
                MB = bcp.tile([128, SQ], f32, tag="bc")
                RS = bcp.tile([128, SQ], f32, tag="bc")
                nc.gpsimd.partition_broadcast(MB[:, 0:W], m1[:, 0:W])
                nc.gpsimd.partition_broadcast(RS[:, 0:W], rstd[:, 0:W])

                def mk(c):
                    return (MB[:, c.start - base:c.stop - base],
                            RS[:, c.start - base:c.stop - base])
                return mk, False

            def ln_norm(mk, in_psum, y, cols, ln_g, ln_b, out_bf,
                        dma_out, fillers=()):
                """Normalize y columns `cols` for all dts.  sub/mul alternate
                DVE / GpSimd per dt (GpSimd cannot read PSUM, so psum-resident
                stats force everything onto DVE); the affine write alternates
                ACT / DVE."""
                W = cols.stop - cols.start
                MB, RS = mk(cols)
                fillers = list(fillers)
                fi = 0
                for dt in range(NDT):
                    t1 = tmpp.tile([128, SQ], f32, tag="lnt")
                    nc.vector.tensor_sub(t1[:, 0:W], y[:, dt, cols], MB)
                    nc.vector.tensor_mul(t1[:, 0:W], t1[:, 0:W], RS)
                    g_ap = ln_g[:, dt:dt + 1]
                    b_ap = ln_b[:, dt:dt + 1]
                    use_act = in_psum or (dt % 2 == 1)
                    if dma_out is not None:
                        od = outp.tile([128, HQ], f32, tag="od")
                        if use_act:
                            nc.scalar.activation(out=od[:, 0:W], in_=t1[:, 0:W],
                                                 func=AF.Identity,
                                                 bias=b_ap, scale=g_ap)
                        else:
                            nc.vector.tensor_scalar(od[:, 0:W], t1[:, 0:W],
                                                    g_ap, b_ap,
                                                    OP.mult, OP.add)
                        nc.sync.dma_start(out=dma_out[dt][:, cols],
                                          in_=od[:, 0:W])
                    else:
                        if use_act:
                            nc.scalar.activation(out=out_bf[:, dt, cols],
                                                 in_=t1[:, 0:W],
                                                 func=AF.Identity,
                                                 bias=b_ap, scale=g_ap)
                        else:
                            nc.vector.tensor_scalar(out_bf[:, dt, cols],
                                                    t1[:, 0:W],
                                                    g_ap, b_ap,
                                                    OP.mult, OP.add)
                    while fi < len(fillers) and fi * NDT < (dt + 1) * len(fillers):
                        fillers[fi]()
                        fi += 1
                while fi < len(fillers):
                    fillers[fi]()
                    fi += 1

            def proj_ln(wo_d, ATTN, bo_sb, resid, ln_g, ln_b, y_tag,
                        out_bf=None, fillers=(),
                        ln_fillers=(), split_cols=False, ln_pe_bcast=False):
                """wo projection + residual + layernorm (feature-major)."""
                y = big.tile([128, NDT, SQ], f32r, tag=y_tag)
                pst1 = p_pav.tile([128, SQ], f32, tag="pavA")
                pst2 = p_pav.tile([128, SQ], f32, tag="pavB")
                fillers = list(fillers)
                fi = 0
                for dt in range(NDT):
                    wot = wcol.tile([128, NDT, 128], bf16, tag="wcol")
                    nc.sync.dma_start(out=wot[:], in_=wo_d[dt])
                    po = p_acc.tile([128, SQ], f32, tag="acc")
                    for ht in range(NDT):
                        nc.tensor.matmul(po[:], wot[:, ht, :], ATTN[:, ht, :],
                                         start=(ht == 0), stop=(ht == NDT - 1))
                    nc.vector.scalar_tensor_tensor(
                        out=y[:, dt, :], in0=po[:], scalar=bo_sb[:, dt:dt + 1],
                        in1=resid[:, dt, :], op0=OP.add, op1=OP.add)
                    sq = tmpp.tile([128, SQ], f32r, tag="sq")
                    nc.scalar.activation(out=sq[:], in_=y[:, dt, :],
                                         func=AF.Square)
                    nc.tensor.matmul(pst1[0:1, :], ones_r[:, 0:1], y[:, dt, :],
                                     start=(dt == 0), stop=(dt == NDT - 1))
                    nc.tensor.matmul(pst2[0:1, :], ones_r[:, 0:1], sq[:],
                                     start=(dt == 0), stop=(dt == NDT - 1))
                    while fi < len(fillers) and fi * NDT < (dt + 1) * len(fillers):
                        fillers[fi]()
                        fi += 1
                while fi < len(fillers):
                    fillers[fi]()
                    fi += 1
                mk, in_psum = ln_stats(pst1, pst2, SQ, 0,
                                       pe_bcast=ln_pe_bcast)
                if split_cols:
                    lf = list(ln_fillers)
                    h = len(lf) // 2
                    ln_norm(mk, in_psum, y, slice(0, HQ), ln_g, ln_b,
                            out_bf, None, fillers=lf[:h])
                    ln_norm(mk, in_psum, y, slice(HQ, SQ), ln_g, ln_b,
                            out_bf, None, fillers=lf[h:])
                else:
                    ln_norm(mk, in_psum, y, slice(0, SQ), ln_g, ln_b,
                            out_bf, None, fillers=ln_fillers)
                return y

            def v3_init(V3A, V3B):
                nc.vector.memset(V3A[:, :, :, 64:65], 1.0)
                nc.vector.memset(V3B[:, :, :, 0:1], 1.0)
                nc.vector.memset(V3B[:, :, :, 1:64], 0.0)

            # ================= self-attention =================
            QT = big.tile([128, NHP, SQ], bf16, tag="qt")
            KT = big.tile([128, NHP, S], bf16, tag="kt")
            V3A = big.tile([128, NKC, NHP, 65], bf16, tag="v3a")
            V3B = big.tile([128, NKC, NHP, 128], bf16, tag="v3b")
            ATTN = big.tile([128, NDT, SQ], bf16, tag="attn")
            v3_init(V3A, V3B)

            # All Q projections first: they only need XQ, so the PE has
            # work while the (larger) XT transfer is still in flight.
            q_steps(0, w_sa_q, XQ, bq1_sb, QT, pre=wq_pre[0])[0]()
            for half in range(2):
                nc.sync.dma_start(out=XT[:, :, 512 * half:512 * (half + 1)],
                                  in_=xt_full[:, :, 512 * half:512 * (half + 1)])
            for hp in range(1, NHP):
                q_steps(hp, w_sa_q, XQ, bq1_sb, QT, pre=wq_pre[hp])[0]()
            sa_k = {hp: k_steps(hp, w_sa_k, XT, KT) for hp in range(NHP)}
            sa_v = {g: v_steps(g, w_sa_v, XT, V3A, V3B) for g in (0, 1)}
            for step in sa_k[0] + sa_v[0]:
                step()
            sa_fill = {
                0: sa_k[1], 1: sa_k[2],
                2: sa_k[3] + sa_v[1][0:4],
                3: sa_k[4] + sa_v[1][4:8],
                4: sa_k[5], 5: sa_k[6], 6: sa_k[7],
            }
            for hp in range(1, NHP):
                attention(hp - 1, QT, KT, V3A, V3B, ATTN, True,
                          sa_fill[hp - 1])

            # ================= cross-attention =================
            # KT2 gets its own slot so the CA K projections can land while
            # the last SA attention is still reading KT.
            KT2 = big.tile([128, NHP, S], bf16, tag="kt2")
            V3A2 = big.tile([128, NKC, NHP, 65], bf16, tag="v3a")
            V3B2 = big.tile([128, NKC, NHP, 128], bf16, tag="v3b")
            QT2 = big.tile([128, NHP, SQ], bf16, tag="qt")
            ATTN2 = big.tile([128, NDT, SQ], bf16, tag="attn")
            # NOTE: no v3_init for V3A2/V3B2 -- they alias V3A/V3B's slots
            # whose ones-column / zero-pad regions are never overwritten.

            # ENC shares XT's slot; the DMA fires as soon as XT is dead
            # (during the last SA attention pair) so CA K/V fillers can
            # start inside the SA tail.  Emitted late so its descriptors
            # don't occupy DMA queue slots during the startup burst.
            ENC = big.tile([128, NDT, S], bf16, tag="xt")
            for half in range(2):
                nc.sync.dma_start(out=ENC[:, :, 512 * half:512 * (half + 1)],
                                  in_=enc_t[:, :, 512 * half:512 * (half + 1)])
            ca_k = {hp: k_steps(hp, w_ca_k, ENC, KT2) for hp in range(NHP)}
            ca_v = {g: v_steps(g, w_ca_v, ENC, V3A2, V3B2) for g in (0, 1)}

            # last SA attention: fill with CA K projections (need only ENC)
            attention(NHP - 1, QT, KT, V3A, V3B, ATTN, True,
                      ca_k[0] + ca_k[1], pe_bcast=True)

            OUT1B = big.tile([128, NDT, SQ], bf16, tag="outb")

            # SA out-proj + LN1: weave the remaining CA K/V work through the
            # projection AND the LN1 normalize loop.  Residuals are the bf16
            # tiles already on chip (XQ here, OUT1B/OUT2B downstream).
            proj_ln(w_sa_o, ATTN, bo1_sb, XQ, ln_sb[1][0], ln_sb[1][1],
                    y_tag="y", out_bf=OUT1B, split_cols=True,
                    fillers=ca_v[0] + ca_k[2] + ca_k[3],
                    ln_fillers=ca_k[4] + ca_k[5] + ca_k[6] + ca_k[7])

            ca_q = {hp: q_steps(hp, w_ca_q, OUT1B, bq2_sb, QT2,
                                halves=(hp == 0))
                    for hp in range(NHP)}
            ca_fill = {
                0: ca_v[1] + ca_q[1],
                1: ca_q[2], 2: ca_q[3],
                3: ca_q[4], 4: ca_q[5],
                5: ca_q[6], 6: ca_q[7],
            }
            for step in ca_q[0]:
                step()
            for hp in range(1, NHP):
                attention(hp - 1, QT2, KT2, V3A2, V3B2, ATTN2, False,
                          ca_fill[hp - 1])
            attention(NHP - 1, QT2, KT2, V3A2, V3B2, ATTN2, False,
                      pe_bcast=True)

            # dt 7 doesn't fit resident: stream it per half in 2KB quarter
            # chunks through the (idle-by-then) wcol pool
            w2q = {}

            def w2_dt7_quarters(h):
                for qq in range(4):
                    t = wcol.tile([128, NDT, 128], bf16, tag="wcol",
                                  name="w2q")
                    nc.sync.dma_start(out=t[:],
                                      in_=w_ff2[7][:, 8 * qq:8 * (qq + 1), :])
                    w2q[(h, qq)] = t

            def w2_lhs(dt, ft, h):
                if dt < 2:
                    return W2kt[:, dt, ft, :]
                if dt < 4:
                    return W2vb[:, dt - 2, ft, :]
                if dt < 7:
                    return W2w[dt - 4][:, ft, :]
                return w2q[(h, ft // 8)][:, ft % 8, :]

            OUT2B = big.tile([128, NDT, SQ], bf16, tag="outb")
            proj_ln(w_ca_o, ATTN2, bo2_sb, OUT1B, ln_sb[2][0], ln_sb[2][1],
                    y_tag="y", out_bf=OUT2B, split_cols=True,
                    ln_pe_bcast=True)

            # ================= feed-forward =================
            # Column-split pipeline: ff1 runs half h0 as soon as LN2's h0
            # columns land, h1 lags LAG f-tiles behind on the same (still
            # resident) weight tile; ff2 + LN3 then run half-by-half so the
            # LN3 tail of h0 hides under the ff2 matmuls of h1.
            H1 = big.tile([128, NFT, SQ], bf16, tag="xt")  # reuse XT slot
            LAG = 4
            w1tiles = {}

            def w1_mm(ft, h):
                cs = slice(HQ * h, HQ * (h + 1))
                ph = p_acc.tile([128, HQ], f32, tag="acc")
                wt = w1tiles[ft]
                for dt in range(NDT):
                    nc.tensor.matmul(ph[:], wt[:, dt, :], OUT2B[:, dt, cs],
                                     start=(dt == 0), stop=(dt == NDT - 1))
                nc.scalar.activation(out=H1[:, ft, cs], in_=ph[:],
                                     func=AF.Relu,
                                     bias=b1_sb[:, ft:ft + 1], scale=1.0)

            for ft in range(NFT):
                wt = wcol.tile([128, NDT, 128], bf16, tag="wcol")
                nc.sync.dma_start(out=wt[:], in_=w_ff1[ft])
                w1tiles[ft] = wt
                w1_mm(ft, 0)
                if ft >= LAG:
                    w1_mm(ft - LAG, 1)
            for ft in range(NFT - LAG, NFT):
                w1_mm(ft, 1)

            # ---- stage ff2 weights into slots freed by the attention ----
            # (kt / v3b slabs are exactly the right size; wbig holds three
            # more dt chunks.)  Emitted after the ff1 weight DMAs so ff1's
            # streaming keeps queue priority; these 8 MiB still have the
            # whole ff1 phase to land.
            W2kt = big.tile([128, 2, NFT, 128], bf16, tag="kt")
            nc.sync.dma_start(out=W2kt[:], in_=w_ff2p[0])
            W2vb = big.tile([128, 2, NFT, 128], bf16, tag="v3b")
            nc.sync.dma_start(out=W2vb[:], in_=w_ff2p[1])
            W2w = []
            for i in range(3):
                t = wbig.tile([128, NFT, 128], bf16, tag="wbig")
                nc.sync.dma_start(out=t[:], in_=w_ff2[4 + i])
                W2w.append(t)

            y3 = big.tile([128, NDT, SQ], f32r, tag="y")

            def w2_half(h):
                cs = slice(HQ * h, HQ * (h + 1))
                pst1 = p_pav.tile([1, HQ], f32, tag="pavA")
                pst2 = p_pav.tile([1, HQ], f32, tag="pavB")
                sqs = {}
                for dt in range(NDT):
                    if dt == 6:
                        w2_dt7_quarters(h)
                    pf = p_acc.tile([128, HQ], f32, tag="acc")
                    for ft in range(NFT):
                        nc.tensor.matmul(pf[:], w2_lhs(dt, ft, h),
                                         H1[:, ft, cs],
                                         start=(ft == 0), stop=(ft == NFT - 1))
                    nc.vector.scalar_tensor_tensor(
                        out=y3[:, dt, cs], in0=pf[:],
                        scalar=b2_sb[:, dt:dt + 1],
                        in1=OUT2B[:, dt, cs], op0=OP.add, op1=OP.add)
                    sq = tmpp.tile([128, HQ], f32r, tag="sq")
                    nc.scalar.activation(out=sq[:], in_=y3[:, dt, cs],
                                         func=AF.Square)
                    sqs[dt] = sq
                    # stats lag one dt so the DVE has a chain-time of slack
                    # before the PE needs its result
                    if dt > 0:
                        nc.tensor.matmul(pst1[0:1, :], ones_r[:, 0:1],
                                         y3[:, dt - 1, cs],
                                         start=(dt == 1), stop=False)
                        nc.tensor.matmul(pst2[0:1, :], ones_r[:, 0:1],
                                         sqs[dt - 1][:],
                                         start=(dt == 1), stop=False)
                nc.tensor.matmul(pst1[0:1, :], ones_r[:, 0:1],
                                 y3[:, NDT - 1, cs], start=False, stop=True)
                nc.tensor.matmul(pst2[0:1, :], ones_r[:, 0:1],
                                 sqs[NDT - 1][:], start=False, stop=True)
                return pst1, pst2

            pst1, pst2 = w2_half(0)
            mk, in_psum = ln_stats(pst1, pst2, HQ, 0)
            ln_norm(mk, in_psum, y3, slice(0, HQ), ln_sb[3][0], ln_sb[3][1],
                    None, out_t)
            pst1, pst2 = w2_half(1)
            mk, in_psum = ln_stats(pst1, pst2, HQ, HQ, pe_bcast=True)
            ln_norm(mk, in_psum, y3, slice(HQ, SQ), ln_sb[3][0], ln_sb[3][1],
                    None, out_t)

    nc.compile()
    return nc


def _qrows(h):
    return np.concatenate(
        [np.arange(64 * (2 * t + h), 64 * (2 * t + h) + 64) for t in range(8)])


def _prepare_in_maps(inputs):
    f = np.float32
    di = np.asarray(inputs["decoder_input"], f)
    eo = np.asarray(inputs["encoder_output"], f)
    mask = np.asarray(inputs["mask"])

    def b16(a):
        return np.ascontiguousarray(a).astype(BF16)

    def wmat(w):  # (H, D, DH) -> (D, H*DH)
        return np.transpose(np.asarray(w, f), (1, 0, 2)).reshape(D, H * DH)

    def colmajor(w, no, co):  # [D_in, N_out] -> [no, 128, D_in/128, co]
        return w.reshape(w.shape[0] // 128, 128, no, co).transpose(2, 1, 0, 3)

    def pmajor(xt, n):  # [D, n] (feature-major) -> [128, NDT, n]
        return np.ascontiguousarray(
            xt.reshape(NDT, 128, n).transpose(1, 0, 2))

    shared = {}
    vecs = {}
    for p in ("sa", "ca"):
        shared[f"w_{p}_q"] = b16(colmajor(wmat(inputs[f"{p}_wq"]), NHP, 128))
        shared[f"w_{p}_k"] = b16(colmajor(wmat(inputs[f"{p}_wk"]), NHP, 128))
        shared[f"w_{p}_v"] = b16(colmajor(wmat(inputs[f"{p}_wv"]), 2, 512))
        wo = np.asarray(inputs[f"{p}_wo"], f)
        shared[f"w_{p}_o"] = b16(colmajor(wo, NDT, 128))
        vecs[f"bq_{p}"] = np.asarray(inputs[f"{p}_bq"], f).reshape(H * DH)
        bv = np.asarray(inputs[f"{p}_bv"], f).reshape(H * DH)
        vecs[f"bo_{p}"] = np.asarray(inputs[f"{p}_bo"], f) + bv @ wo
    shared["w_ff1"] = b16(colmajor(np.asarray(inputs["ff_w1"], f), NFT, 128))
    w2cm = b16(colmajor(np.asarray(inputs["ff_w2"], f), NDT, 128))
    shared["w_ff2"] = w2cm
    # dt pairs (0,1) and (2,3) pre-packed as [2][128, 2, NFT, 128]
    shared["w_ff2p"] = np.ascontiguousarray(
        w2cm[0:4].reshape(2, 2, 128, NFT, 128).transpose(0, 2, 1, 3, 4))

    def cols(v, n):  # [n*128] -> [128, n]
        return np.asarray(v, f).reshape(n, 128).T

    va = np.concatenate([
        cols(vecs["bq_sa"], NHP), cols(vecs["bq_ca"], NHP),
        cols(vecs["bo_sa"], NDT), cols(vecs["bo_ca"], NDT),
        cols(inputs["ff_b2"], NDT),
        cols(inputs["ln1_g"], NDT), cols(inputs["ln1_b"], NDT),
        cols(inputs["ln2_g"], NDT), cols(inputs["ln2_b"], NDT),
        cols(inputs["ln3_g"], NDT), cols(inputs["ln3_b"], NDT),
        cols(inputs["ff_b1"], NFT),
    ], axis=1)
    shared["v_all"] = np.ascontiguousarray(va, dtype=f)

    qr = {h: _qrows(h) for h in (0, 1)}
    in_maps = []
    for c in range(NCORES):
        b, h = divmod(c, 2)
        X = di[b]
        m = dict(shared)
        m["xt_full"] = b16(pmajor(X.T, S))
        Xq = X[qr[h]]
        m["xq"] = b16(pmajor(Xq.T, SQ))
        m["enc_t"] = b16(pmajor(eo[b].T, S))
        mb = mask[b][qr[h]].astype(f)          # [SQ q, S k]
        slabs = np.zeros((NKC, 128, 64), f)
        for j in range(NKC):
            slabs[j] = mb[64 * j:64 * j + 64, 128 * j:128 * (j + 1)].T
        m["sa_mask"] = np.ascontiguousarray(
            slabs.transpose(1, 0, 2)).astype(BF16)
        in_maps.append(m)
    return in_maps


def _collect_output(results):
    qr = {h: _qrows(h) for h in (0, 1)}
    out = np.zeros((B, S, D), np.float32)
    for c in range(NCORES):
        b, h = divmod(c, 2)
        ot = np.asarray(results[c]["out_t"], np.float32).reshape(D, SQ)
        out[b, qr[h]] = ot.T
    return out


def kernel(**inputs):
    global _PROG
    if _PROG is None:
        _PROG = _build_program()
    from concourse.bass_utils import run_bass_kernel_spmd

    in_maps = _prepare_in_maps(inputs)
    res = run_bass_kernel_spmd(_PROG, in_maps, list(range(NCORES)))
    if res.exec_time_ns is not None:
        print(f"HW exec time: {res.exec_time_ns} ns")
    return _collect_output(res.results)


# revision 24
# speedup vs baseline: 1.1880x; 1.1880x over previous
"""Trainium2 Bass kernel for a transformer decoder block.

Shapes (hardcoded): B=4, S=1024, D=1024, H=16 heads, DH=64, FFN F=4096.

Sharding: 8 cores = 4 batches x 2 sequence-halves.  Core (b, h) handles
query rows {64*(2t+h)+r : t in 0..7, r in 0..63} of batch b (interleaved
64-row blocks so the causal-attention work per core is identical -> one
uniform SPMD program).  Each core recomputes the (small) K/V projections
it needs, so no collectives are required.

On-chip layout is feature-major ("transposed"): activations live as
[feature, token] so every matmul contraction sits on the partition axis.
The host pre-transposes inputs/weights and re-transposes the output.

Scheduling: engines execute their instruction streams in order, so each
attention head-pair's softmax (ScalarE-bound) is emitted with "filler"
projection matmul groups woven between its k-chunks, keeping the PE busy
while exps drain.  The FFN + final layernorm run in column halves so the
LN2/LN3 tails overlap FFN matmuls, with the ff2 weights held resident in
slots freed by the attention phase.
"""

import sys

if "/opt/trn_rl_repo" not in sys.path:
    sys.path.insert(0, "/opt/trn_rl_repo")

import numpy as np
import ml_dtypes

B, S, D, H, F, DH = 4, 1024, 1024, 16, 4096, 64
NCORES = 8
SQ = 512            # query rows per core
HQ = 256            # half of SQ (FFN column split)
NDT = D // 128      # 8 d-tiles
NFT = F // 128      # 32 f-tiles
NHP = H // 2        # 8 head pairs
NKC = S // 128      # 8 k chunks
BF16 = ml_dtypes.bfloat16

_PROG = None


def _build_program():
    import concourse.mybir as mybir
    from concourse import bacc
    from concourse.tile import TileContext

    f32 = mybir.dt.float32
    bf16 = mybir.dt.bfloat16
    f32r = mybir.dt.float32r
    AF = mybir.ActivationFunctionType
    OP = mybir.AluOpType

    nc = bacc.Bacc("TRN2", target_bir_lowering=False, debug=False,
                   num_devices=NCORES)

    def din(name, shape, dt=bf16):
        return nc.dram_tensor(name, shape, dt, kind="ExternalInput")

    # activations, partition-major so each loads with few contiguous DMAs
    xt_full = din("xt_full", [128, NDT, S])          # X^T (K/V source)
    xq = din("xq", [128, NDT, SQ])                   # X^T own q rows
    enc_t = din("enc_t", [128, NDT, S])              # encoder^T
    sa_mask = din("sa_mask", [128, NKC, 64])         # causal boundary slabs

    # weights staged host-side in exactly the sbuf tile layout
    w_sa_q = din("w_sa_q", [NHP, 128, NDT, 128])
    w_sa_k = din("w_sa_k", [NHP, 128, NDT, 128])
    w_sa_v = din("w_sa_v", [2, 128, NDT, 512])
    w_sa_o = din("w_sa_o", [NDT, 128, NDT, 128])
    w_ca_q = din("w_ca_q", [NHP, 128, NDT, 128])
    w_ca_k = din("w_ca_k", [NHP, 128, NDT, 128])
    w_ca_v = din("w_ca_v", [2, 128, NDT, 512])
    w_ca_o = din("w_ca_o", [NDT, 128, NDT, 128])
    w_ff1 = din("w_ff1", [NFT, 128, NDT, 128])
    w_ff2 = din("w_ff2", [NDT, 128, NFT, 128])
    # ff2 dt-pairs pre-packed [128, 2, NFT, 128] for the resident slabs
    w_ff2p = din("w_ff2p", [2, 128, 2, NFT, 128])

    # all small per-feature vectors concatenated: one DMA
    # cols: bq1 0:8 | bq2 8:16 | bo1 16:24 | bo2 24:32 | b2 32:40 |
    #       ln1g 40:48 | ln1b 48:56 | ln2g .. | ln3b 72:88 | b1 88:120
    NV = 120
    v_all = din("v_all", [128, NV], f32)

    out_t = nc.dram_tensor("out_t", [NDT, 128, SQ], f32, kind="ExternalOutput")

    with TileContext(nc) as tc:
        with tc.tile_pool(name="p_acc", bufs=2, space="PSUM") as p_acc, \
             tc.tile_pool(name="p_s", bufs=2, space="PSUM") as p_s, \
             tc.tile_pool(name="p_pav", bufs=1, space="PSUM") as p_pav, \
             tc.tile_pool(name="const", bufs=1) as cpool, \
             tc.tile_pool(name="big", bufs=1) as big, \
             tc.tile_pool(name="wcol", bufs=8) as wcol, \
             tc.tile_pool(name="wbig", bufs=3) as wbig, \
             tc.tile_pool(name="pt", bufs=2) as ptp, \
             tc.tile_pool(name="bc", bufs=4) as bcp, \
             tc.tile_pool(name="sm", bufs=1) as smp, \
             tc.tile_pool(name="tmp", bufs=2) as tmpp, \
             tc.tile_pool(name="outp", bufs=2) as outp:

            # ---------------- activation loads first (startup latency) ----
            # first XQ chunk and the first q weight lead the DMA queues so
            # the first matmul chain can start as early as possible
            XQ = big.tile([128, NDT, SQ], bf16, tag="outb")
            nc.sync.dma_start(out=XQ[:, 0:2, :], in_=xq[:, 0:2, :])
            wq_pre = []
            for hp in range(NHP):
                t = wcol.tile([128, NDT, 128], bf16, tag="wcol", name="wqt")
                nc.sync.dma_start(out=t[:], in_=w_sa_q[hp])
                wq_pre.append(t)
            for c in range(1, 4):
                nc.sync.dma_start(out=XQ[:, 2 * c:2 * c + 2, :],
                                  in_=xq[:, 2 * c:2 * c + 2, :])

            # ---------------- constants / small vectors ----------------
            # LN stat matmuls use 1/D so psum rows are mean / E[x^2] directly
            oned_f = cpool.tile([128, 1], f32)
            nc.vector.memset(oned_f[:], 1.0 / D)
            ones_r = cpool.tile([128, 1], f32r)
            nc.vector.tensor_copy(ones_r[:], oned_f[:])
            ones_pe = cpool.tile([1, 128], f32)
            nc.vector.memset(ones_pe[:], 1.0)
            ones_col = cpool.tile([128, 128], bf16)
            nc.vector.memset(ones_col[:], 1.0)
            eps_t = cpool.tile([1, 1], f32)
            nc.vector.memset(eps_t[:], 1e-12)

            VA = cpool.tile([128, NV], f32)
            nc.sync.dma_start(out=VA[:], in_=v_all[:])
            bq1_sb, bq2_sb = VA[:, 0:8], VA[:, 8:16]
            bo1_sb, bo2_sb = VA[:, 16:24], VA[:, 24:32]
            b2_sb = VA[:, 32:40]
            ln_sb = {j: (VA[:, 40 + 16 * (j - 1):48 + 16 * (j - 1)],
                         VA[:, 48 + 16 * (j - 1):56 + 16 * (j - 1)])
                     for j in (1, 2, 3)}
            b1_sb = VA[:, 88:120]

            MS = cpool.tile([128, NKC, 64], bf16)
            nc.sync.dma_start(out=MS[:], in_=sa_mask[:])

            XT = big.tile([128, NDT, S], bf16, tag="xt")

            # ---------------- filler-step builders ----------------
            # Each returned closure emits one psum matmul group; they are
            # woven between attention k-chunks to keep the PE fed while the
            # ScalarE runs the softmax exps.
            def q_steps(hp, wq_d, src_q, bq_sb, QT, halves=False,
                        pre=None):
                cell = {"w": pre}

                def run_h(cs):
                    def run():
                        if cell["w"] is None:
                            cell["w"] = wcol.tile([128, NDT, 128], bf16,
                                                  tag="wcol", name="wqt")
                            nc.sync.dma_start(out=cell["w"][:], in_=wq_d[hp])
                        wqt = cell["w"]
                        W = cs.stop - cs.start
                        pq = p_acc.tile([128, SQ], f32, tag="acc")
                        for dt in range(NDT):
                            nc.tensor.matmul(pq[:, 0:W], wqt[:, dt, :],
                                             src_q[:, dt, cs],
                                             start=(dt == 0),
                                             stop=(dt == NDT - 1))
                        nc.vector.tensor_scalar_add(QT[:, hp, cs], pq[:, 0:W],
                                                    bq_sb[:, hp:hp + 1])
                    return run
                if halves:
                    return [run_h(slice(0, HQ)), run_h(slice(HQ, SQ))]
                return [run_h(slice(0, SQ))]

            def k_steps(hp, wk_d, src_kv, KT):
                cell = {}

                def run_kh(kh):
                    def run():
                        if kh == 0:
                            cell["w"] = wcol.tile([128, NDT, 128], bf16,
                                                  tag="wcol", name="wkt")
                            nc.sync.dma_start(out=cell["w"][:], in_=wk_d[hp])
                        wkt = cell["w"]
                        pk = p_acc.tile([128, 512], f32, tag="acc")
                        for dt in range(NDT):
                            nc.tensor.matmul(
                                pk[:], wkt[:, dt, :],
                                src_kv[:, dt, 512 * kh:512 * (kh + 1)],
                                start=(dt == 0), stop=(dt == NDT - 1))
                        nc.vector.tensor_copy(
                            KT[:, hp, 512 * kh:512 * (kh + 1)], pk[:])
                    return run
                return [run_kh(0), run_kh(1)]

            def v_steps(g, wv_d, src_kv, V3A, V3B):
                cell = {}

                def run_kc(kc):
                    def run():
                        if kc == 0:
                            cell["w"] = wbig.tile([128, NDT, 512], bf16,
                                                  tag="wbig", name="wvt")
                            nc.sync.dma_start(out=cell["w"][:], in_=wv_d[g])
                        wvt = cell["w"]
                        pv = p_acc.tile([128, 4, 128], f32, tag="acc")
                        for dt in range(NDT):
                            nc.tensor.matmul(
                                pv[:, :, :],
                                src_kv[:, dt, 128 * kc:128 * (kc + 1)],
                                wvt[:, dt, :],
                                start=(dt == 0), stop=(dt == NDT - 1))
                        nc.vector.tensor_copy(
                            V3A[:, kc, 4 * g:4 * g + 4, 0:64], pv[:, :, 0:64])
                        nc.vector.tensor_copy(
                            V3B[:, kc, 4 * g:4 * g + 4, 64:128],
                            pv[:, :, 64:128])
                    return run
                return [run_kc(kc) for kc in range(NKC)]

            def attention(hp, QT, KT, V3A, V3B, ATTN, causal, fillers=(),
                          pe_bcast=False):
                # V3A head slab = [V_A(64) | ones] -> AV rows 0:64, denom row
                # 64.  V3B = [ones | pad(63) | V_B(64)] -> denom row 0, AV
                # rows 64:128.  The ones column makes the AV matmul emit the
                # softmax denominator for free (no separate 1-row matmuls).
                # Both heads' scores share ONE 2-bank psum tile so each chunk
                # needs a single (strided) exp activation.
                pavA = p_pav.tile([128, SQ], f32, tag="pavA")
                pavB = p_pav.tile([128, SQ], f32, tag="pavB")
                fillers = list(fillers)
                fi = 0
                for j in range(NKC):
                    n0 = 64 * j if causal else 0
                    s_ = p_s.tile([128, 2, SQ], f32, tag="s")
                    ks = slice(128 * j, 128 * (j + 1))
                    nc.tensor.matmul(s_[:, 0, n0:SQ], KT[0:64, hp, ks],
                                     QT[0:64, hp, n0:SQ], start=True, stop=True)
                    nc.tensor.matmul(s_[:, 1, n0:SQ], KT[64:128, hp, ks],
                                     QT[64:128, hp, n0:SQ], start=True,
                                     stop=True)
                    pt = ptp.tile([128, 2, SQ], bf16, tag="pt")
                    nc.scalar.activation(out=pt[:, :, n0:SQ],
                                         in_=s_[:, :, n0:SQ],
                                         func=AF.Exp, scale=0.125)
                    if causal:
                        nc.vector.tensor_mul(pt[:, 0, n0:n0 + 64],
                                             pt[:, 0, n0:n0 + 64], MS[:, j, :])
                        nc.vector.tensor_mul(pt[:, 1, n0:n0 + 64],
                                             pt[:, 1, n0:n0 + 64], MS[:, j, :])
                    # fillers go HERE (between scores and AV) so the PE chews
                    # on them while ScalarE exps this chunk
                    while fi < len(fillers) and fi * NKC < (j + 1) * len(fillers):
                        fillers[fi]()
                        fi += 1
                    st, sp = (j == 0), (j == NKC - 1)
                    nc.tensor.matmul(pavA[0:65, n0:SQ],
                                     V3A[:, j, hp, 0:65],
                                     pt[:, 0, n0:SQ], start=st, stop=sp)
                    nc.tensor.matmul(pavB[:, n0:SQ],
                                     V3B[:, j, hp, :],
                                     pt[:, 1, n0:SQ], start=st, stop=sp)
                while fi < len(fillers):
                    fillers[fi]()
                    fi += 1
                if pe_bcast:
                    # Final attention of a phase: the next pav user (the
                    # projection's stat tiles) has slack, so skip the full
                    # evacuation.  Copy just the two denominator rows to
                    # SBUF, broadcast them raw with 1-contraction matmuls
                    # (the PE is idle here), invert the broadcast, and
                    # normalize straight out of PSUM (one psum operand).
                    dn = bcp.tile([128, SQ], bf16, tag="dn")
                    nc.scalar.activation(out=dn[64:65, :], in_=pavA[64:65, :],
                                         func=AF.Identity, scale=1.0)
                    nc.scalar.activation(out=dn[0:1, :], in_=pavB[0:1, :],
                                         func=AF.Identity, scale=1.0)
                    pd = p_s.tile([128, 2, SQ], f32, tag="s")
                    nc.tensor.matmul(pd[:, 0, :], ones_col[64:65, :],
                                     dn[64:65, :], start=True, stop=True)
                    nc.tensor.matmul(pd[:, 1, :], ones_col[0:1, :],
                                     dn[0:1, :], start=True, stop=True)
                    rA = bcp.tile([128, SQ], f32, tag="bc")
                    rB = bcp.tile([128, SQ], f32, tag="bc")
                    nc.vector.reciprocal_approx_fast(out=rA[:], in_=pd[:, 0, :])
                    nc.vector.reciprocal_approx_fast(out=rB[:], in_=pd[:, 1, :])
                    nc.vector.tensor_mul(ATTN[0:64, hp, :], pavA[0:64, :],
                                         rA[0:64, :])
                    nc.vector.tensor_mul(ATTN[64:128, hp, :], pavB[64:128, :],
                                         rB[64:128, :])
                    return
                # Evacuate the accumulators to SBUF immediately so the pav
                # psum banks free up for the next attention (p_pav bufs=1).
                # partition_broadcast reads only partition 0; tensor_copy can
                # shift partitions sbuf->sbuf, so: copy out, shift the denom
                # rows to partition 0, recip, broadcast, normalize.
                cpA = bcp.tile([128, SQ], f32, tag="bc")
                cpB = bcp.tile([128, SQ], f32, tag="bc")
                nc.scalar.activation(out=cpA[0:65, :], in_=pavA[0:65, :],
                                     func=AF.Identity, scale=1.0)
                nc.scalar.activation(out=cpB[:, :], in_=pavB[:, :],
                                     func=AF.Identity, scale=1.0)
                ra = smp.tile([1, SQ], f32, tag="ra")
                rb = smp.tile([1, SQ], f32, tag="rb")
                nc.vector.tensor_copy(ra[:], cpA[64:65, :])
                nc.vector.reciprocal_approx_fast(out=ra[:], in_=ra[:])
                nc.vector.tensor_copy(rb[:], cpB[0:1, :])
                nc.vector.reciprocal_approx_fast(out=rb[:], in_=rb[:])
                DAt = bcp.tile([128, SQ], f32, tag="bc")
                DBt = bcp.tile([128, SQ], f32, tag="bc")
                nc.gpsimd.partition_broadcast(DAt[:], ra[:])
                nc.gpsimd.partition_broadcast(DBt[:], rb[:])
                nc.vector.tensor_mul(ATTN[0:64, hp, :], cpA[0:64, :],
                                     DAt[0:64, :])
                nc.vector.tensor_mul(ATTN[64:128, hp, :], cpB[64:128, :],
                                     DBt[64:128, :])

            def ln_stats(pst1, pst2, W, base, pe_bcast=False):
                """Turn accumulated sum / sum-sq psum rows (local cols 0:W,
                representing global cols base:base+W) into broadcast mean +
                rstd.  Returns (mk, in_psum): mk(global col slice) ->
                (mean_ap, rstd_ap)."""
                m1 = smp.tile([1, SQ], f32, tag="m1")
                nc.vector.tensor_copy(m1[:, 0:W], pst1[0:1, 0:W])
                sq1 = smp.tile([1, SQ], f32, tag="ra")
                nc.scalar.activation(out=sq1[:, 0:W], in_=pst1[0:1, 0:W],
                                     func=AF.Square)
                varp = smp.tile([1, SQ], f32, tag="varp")
                nc.vector.tensor_sub(varp[:, 0:W], pst2[0:1, 0:W],
                                     sq1[:, 0:W])
                sv = smp.tile([1, SQ], f32, tag="rb")
                nc.scalar.activation(out=sv[:, 0:W], in_=varp[:, 0:W],
                                     func=AF.Sqrt, bias=eps_t[:],
                                     scale=float(D) / (D - 1))
                rstd = smp.tile([1, SQ], f32, tag="rstd")
                nc.vector.reciprocal_approx_fast(out=rstd[:, 0:W],
                                                 in_=sv[:, 0:W])
                if pe_bcast:
                    # PE is otherwise idle (end of kernel): broadcast via a
                    # 1-contraction matmul instead of the slower gpsimd path.
                    pd = p_s.tile([128, 2, SQ], f32, tag="s")
                    nc.tensor.matmul(pd[:, 0, 0:W], ones_pe[:], m1[:, 0:W],
                                     start=True, stop=True)
                    nc.tensor.matmul(pd[:, 1, 0:W], ones_pe[:], rstd[:, 0:W],
                                     start=True, stop=True)

                    def mk(c):
                        return (pd[:, 0, c.start - base:c.stop - base],
                                pd[:, 1, c.start - base:c.stop - base])
                    return mk, True
                MB = bcp.tile([128, SQ], f32, tag="bc")
                RS = bcp.tile([128, SQ], f32, tag="bc")
                nc.gpsimd.partition_broadcast(MB[:, 0:W], m1[:, 0:W])
                nc.gpsimd.partition_broadcast(RS[:, 0:W], rstd[:, 0:W])

                def mk(c):
                    return (MB[:, c.start - base:c.stop - base],
                            RS[:, c.start - base:c.stop - base])
                return mk, False

            def ln_norm(mk, in_psum, y, cols, ln_g, ln_b, out_bf,
                        dma_out, fillers=()):
                """Normalize y columns `cols` for all dts.  sub/mul alternate
                DVE / GpSimd per dt (GpSimd cannot read PSUM, so psum-resident
                stats force everything onto DVE); the affine write alternates
                ACT / DVE."""
                W = cols.stop - cols.start
                MB, RS = mk(cols)
                fillers = list(fillers)
                fi = 0
                for dt in range(NDT):
                    t1 = tmpp.tile([128, SQ], f32, tag="lnt")
                    nc.vector.tensor_sub(t1[:, 0:W], y[:, dt, cols], MB)
                    nc.vector.tensor_mul(t1[:, 0:W], t1[:, 0:W], RS)
                    g_ap = ln_g[:, dt:dt + 1]
                    b_ap = ln_b[:, dt:dt + 1]
                    use_act = in_psum or (dt % 2 == 1)
                    if dma_out is not None:
                        od = outp.tile([128, HQ], f32, tag="od")
                        if use_act:
                            nc.scalar.activation(out=od[:, 0:W], in_=t1[:, 0:W],
                                                 func=AF.Identity,
                                                 bias=b_ap, scale=g_ap)
                        else:
                            nc.vector.tensor_scalar(od[:, 0:W], t1[:, 0:W],
                                                    g_ap, b_ap,
                                                    OP.mult, OP.add)
                        nc.sync.dma_start(out=dma_out[dt][:, cols],
                                          in_=od[:, 0:W])
                    else:
                        if use_act:
                            nc.scalar.activation(out=out_bf[:, dt, cols],
                                                 in_=t1[:, 0:W],
                                                 func=AF.Identity,
                                                 bias=b_ap, scale=g_ap)
                        else:
                            nc.vector.tensor_scalar(out_bf[:, dt, cols],
                                                    t1[:, 0:W],
                                                    g_ap, b_ap,
                                                    OP.mult, OP.add)
                    while fi < len(fillers) and fi * NDT < (dt + 1) * len(fillers):
                        fillers[fi]()
                        fi += 1
                while fi < len(fillers):
                    fillers[fi]()
                    fi += 1

            def proj_ln(wo_d, ATTN, bo_sb, resid, ln_g, ln_b, y_tag,
                        out_bf=None, fillers=(),
                        ln_fillers=(), split_cols=False, ln_pe_bcast=False):
                """wo projection + residual + layernorm (feature-major)."""
                y = big.tile([128, NDT, SQ], f32r, tag=y_tag)
                pst1 = p_pav.tile([128, SQ], f32, tag="pavA")
                pst2 = p_pav.tile([128, SQ], f32, tag="pavB")
                fillers = list(fillers)
                fi = 0
                ys = {}
                for dt in range(NDT):
                    wot = wcol.tile([128, NDT, 128], bf16, tag="wcol")
                    nc.sync.dma_start(out=wot[:], in_=wo_d[dt])
                    po = p_acc.tile([128, SQ], f32, tag="acc")
                    for ht in range(NDT):
                        nc.tensor.matmul(po[:], wot[:, ht, :], ATTN[:, ht, :],
                                         start=(ht == 0), stop=(ht == NDT - 1))
                    # evacuate via ACT so the psum bank frees without waiting
                    # on the (possibly backlogged) DVE queue
                    pc = tmpp.tile([128, SQ], f32, tag="lnt")
                    nc.scalar.activation(out=pc[:], in_=po[:],
                                         func=AF.Identity, scale=1.0)
                    nc.vector.scalar_tensor_tensor(
                        out=y[:, dt, :], in0=pc[:], scalar=bo_sb[:, dt:dt + 1],
                        in1=resid[:, dt, :], op0=OP.add, op1=OP.add)
                    sq = tmpp.tile([128, SQ], f32r, tag="sq")
                    nc.scalar.activation(out=sq[:], in_=y[:, dt, :],
                                         func=AF.Square)
                    ys[dt] = sq
                    # stats lag one dt so the PE never waits on fresh DVE/ACT
                    # output
                    if dt > 0:
                        nc.tensor.matmul(pst1[0:1, :], ones_r[:, 0:1],
                                         y[:, dt - 1, :],
                                         start=(dt == 1), stop=False)
                        nc.tensor.matmul(pst2[0:1, :], ones_r[:, 0:1],
                                         ys[dt - 1][:],
                                         start=(dt == 1), stop=False)
                    while fi < len(fillers) and fi * NDT < (dt + 1) * len(fillers):
                        fillers[fi]()
                        fi += 1
                while fi < len(fillers):
                    fillers[fi]()
                    fi += 1
                nc.tensor.matmul(pst1[0:1, :], ones_r[:, 0:1],
                                 y[:, NDT - 1, :], start=False, stop=True)
                nc.tensor.matmul(pst2[0:1, :], ones_r[:, 0:1],
                                 ys[NDT - 1][:], start=False, stop=True)
                mk, in_psum = ln_stats(pst1, pst2, SQ, 0,
                                       pe_bcast=ln_pe_bcast)
                if split_cols:
                    lf = list(ln_fillers)
                    h = len(lf) // 2
                    ln_norm(mk, in_psum, y, slice(0, HQ), ln_g, ln_b,
                            out_bf, None, fillers=lf[:h])
                    ln_norm(mk, in_psum, y, slice(HQ, SQ), ln_g, ln_b,
                            out_bf, None, fillers=lf[h:])
                else:
                    ln_norm(mk, in_psum, y, slice(0, SQ), ln_g, ln_b,
                            out_bf, None, fillers=ln_fillers)
                return y

            def v3_init(V3A, V3B):
                nc.vector.memset(V3A[:, :, :, 64:65], 1.0)
                nc.vector.memset(V3B[:, :, :, 0:1], 1.0)
                nc.vector.memset(V3B[:, :, :, 1:64], 0.0)

            # ================= self-attention =================
            QT = big.tile([128, NHP, SQ], bf16, tag="qt")
            KT = big.tile([128, NHP, S], bf16, tag="kt")
            V3A = big.tile([128, NKC, NHP, 65], bf16, tag="v3a")
            V3B = big.tile([128, NKC, NHP, 128], bf16, tag="v3b")
            ATTN = big.tile([128, NDT, SQ], bf16, tag="attn")
            v3_init(V3A, V3B)

            # All Q projections first: they only need XQ, so the PE has
            # work while the (larger) XT transfer is still in flight.
            q_steps(0, w_sa_q, XQ, bq1_sb, QT, pre=wq_pre[0])[0]()
            for half in range(2):
                nc.sync.dma_start(out=XT[:, :, 512 * half:512 * (half + 1)],
                                  in_=xt_full[:, :, 512 * half:512 * (half + 1)])
            for hp in range(1, NHP):
                q_steps(hp, w_sa_q, XQ, bq1_sb, QT, pre=wq_pre[hp])[0]()
            sa_k = {hp: k_steps(hp, w_sa_k, XT, KT) for hp in range(NHP)}
            sa_v = {g: v_steps(g, w_sa_v, XT, V3A, V3B) for g in (0, 1)}
            for step in sa_k[0] + sa_v[0]:
                step()
            sa_fill = {
                0: sa_k[1], 1: sa_k[2],
                2: sa_k[3] + sa_v[1][0:4],
                3: sa_k[4] + sa_v[1][4:8],
                4: sa_k[5], 5: sa_k[6], 6: sa_k[7],
            }
            for hp in range(1, NHP):
                attention(hp - 1, QT, KT, V3A, V3B, ATTN, True,
                          sa_fill[hp - 1])

            # ================= cross-attention =================
            # KT2 gets its own slot so the CA K projections can land while
            # the last SA attention is still reading KT.
            KT2 = big.tile([128, NHP, S], bf16, tag="kt2")
            V3A2 = big.tile([128, NKC, NHP, 65], bf16, tag="v3a")
            V3B2 = big.tile([128, NKC, NHP, 128], bf16, tag="v3b")
            QT2 = big.tile([128, NHP, SQ], bf16, tag="qt")
            ATTN2 = big.tile([128, NDT, SQ], bf16, tag="attn")
            # NOTE: no v3_init for V3A2/V3B2 -- they alias V3A/V3B's slots
            # whose ones-column / zero-pad regions are never overwritten.

            # ENC shares XT's slot; the DMA fires as soon as XT is dead
            # (during the last SA attention pair) so CA K/V fillers can
            # start inside the SA tail.  Emitted late so its descriptors
            # don't occupy DMA queue slots during the startup burst.
            ENC = big.tile([128, NDT, S], bf16, tag="xt")
            for half in range(2):
                nc.sync.dma_start(out=ENC[:, :, 512 * half:512 * (half + 1)],
                                  in_=enc_t[:, :, 512 * half:512 * (half + 1)])
            ca_k = {hp: k_steps(hp, w_ca_k, ENC, KT2) for hp in range(NHP)}
            ca_v = {g: v_steps(g, w_ca_v, ENC, V3A2, V3B2) for g in (0, 1)}

            # last SA attention: fill with CA K projections (need only ENC)
            attention(NHP - 1, QT, KT, V3A, V3B, ATTN, True,
                      ca_k[0] + ca_k[1], pe_bcast=True)

            OUT1B = big.tile([128, NDT, SQ], bf16, tag="outb")

            # SA out-proj + LN1: weave the remaining CA K/V work through the
            # projection AND the LN1 normalize loop.  Residuals are the bf16
            # tiles already on chip (XQ here, OUT1B/OUT2B downstream).
            proj_ln(w_sa_o, ATTN, bo1_sb, XQ, ln_sb[1][0], ln_sb[1][1],
                    y_tag="y", out_bf=OUT1B, split_cols=True,
                    fillers=ca_v[0] + ca_k[2] + ca_k[3],
                    ln_fillers=ca_k[4] + ca_k[5] + ca_k[6] + ca_k[7])

            ca_q = {hp: q_steps(hp, w_ca_q, OUT1B, bq2_sb, QT2,
                                halves=(hp == 0))
                    for hp in range(NHP)}
            ca_fill = {
                0: ca_v[1] + ca_q[1],
                1: ca_q[2], 2: ca_q[3],
                3: ca_q[4], 4: ca_q[5],
                5: ca_q[6], 6: ca_q[7],
            }
            for step in ca_q[0]:
                step()
            for hp in range(1, NHP):
                attention(hp - 1, QT2, KT2, V3A2, V3B2, ATTN2, False,
                          ca_fill[hp - 1])
            attention(NHP - 1, QT2, KT2, V3A2, V3B2, ATTN2, False,
                      pe_bcast=True)

            # dt 7 doesn't fit resident: stream it per half in 2KB quarter
            # chunks through the (idle-by-then) wcol pool
            w2q = {}

            def w2_dt7_quarters(h):
                for qq in range(4):
                    t = wcol.tile([128, NDT, 128], bf16, tag="wcol",
                                  name="w2q")
                    nc.sync.dma_start(out=t[:],
                                      in_=w_ff2[7][:, 8 * qq:8 * (qq + 1), :])
                    w2q[(h, qq)] = t

            def w2_lhs(dt, ft, h):
                if dt < 2:
                    return W2kt[:, dt, ft, :]
                if dt < 4:
                    return W2vb[:, dt - 2, ft, :]
                if dt < 7:
                    return W2w[dt - 4][:, ft, :]
                return w2q[(h, ft // 8)][:, ft % 8, :]

            OUT2B = big.tile([128, NDT, SQ], bf16, tag="outb")
            proj_ln(w_ca_o, ATTN2, bo2_sb, OUT1B, ln_sb[2][0], ln_sb[2][1],
                    y_tag="y", out_bf=OUT2B, split_cols=True,
                    ln_pe_bcast=True)

            # ================= feed-forward =================
            # Column-split pipeline: ff1 runs half h0 as soon as LN2's h0
            # columns land, h1 lags LAG f-tiles behind on the same (still
            # resident) weight tile; ff2 + LN3 then run half-by-half so the
            # LN3 tail of h0 hides under the ff2 matmuls of h1.
            H1 = big.tile([128, NFT, SQ], bf16, tag="xt")  # reuse XT slot
            LAG = 4
            w1tiles = {}

            def w1_mm(ft, h):
                cs = slice(HQ * h, HQ * (h + 1))
                ph = p_acc.tile([128, HQ], f32, tag="acc")
                wt = w1tiles[ft]
                for dt in range(NDT):
                    nc.tensor.matmul(ph[:], wt[:, dt, :], OUT2B[:, dt, cs],
                                     start=(dt == 0), stop=(dt == NDT - 1))
                nc.scalar.activation(out=H1[:, ft, cs], in_=ph[:],
                                     func=AF.Relu,
                                     bias=b1_sb[:, ft:ft + 1], scale=1.0)

            for ft in range(NFT):
                wt = wcol.tile([128, NDT, 128], bf16, tag="wcol")
                nc.sync.dma_start(out=wt[:], in_=w_ff1[ft])
                w1tiles[ft] = wt
                w1_mm(ft, 0)
                if ft >= LAG:
                    w1_mm(ft - LAG, 1)
            for ft in range(NFT - LAG, NFT):
                w1_mm(ft, 1)

            # ---- stage ff2 weights into slots freed by the attention ----
            # (kt / v3b slabs are exactly the right size; wbig holds three
            # more dt chunks.)  Emitted after the ff1 weight DMAs so ff1's
            # streaming keeps queue priority; these 8 MiB still have the
            # whole ff1 phase to land.
            W2kt = big.tile([128, 2, NFT, 128], bf16, tag="kt")
            nc.sync.dma_start(out=W2kt[:], in_=w_ff2p[0])
            W2vb = big.tile([128, 2, NFT, 128], bf16, tag="v3b")
            nc.sync.dma_start(out=W2vb[:], in_=w_ff2p[1])
            W2w = []
            for i in range(3):
                t = wbig.tile([128, NFT, 128], bf16, tag="wbig")
                nc.sync.dma_start(out=t[:], in_=w_ff2[4 + i])
                W2w.append(t)

            y3 = big.tile([128, NDT, SQ], f32r, tag="y")

            def w2_half(h):
                cs = slice(HQ * h, HQ * (h + 1))
                pst1 = p_pav.tile([1, HQ], f32, tag="pavA")
                pst2 = p_pav.tile([1, HQ], f32, tag="pavB")
                sqs = {}
                for dt in range(NDT):
                    if dt == 6:
                        w2_dt7_quarters(h)
                    pf = p_acc.tile([128, HQ], f32, tag="acc")
                    for ft in range(NFT):
                        nc.tensor.matmul(pf[:], w2_lhs(dt, ft, h),
                                         H1[:, ft, cs],
                                         start=(ft == 0), stop=(ft == NFT - 1))
                    nc.vector.scalar_tensor_tensor(
                        out=y3[:, dt, cs], in0=pf[:],
                        scalar=b2_sb[:, dt:dt + 1],
                        in1=OUT2B[:, dt, cs], op0=OP.add, op1=OP.add)
                    sq = tmpp.tile([128, HQ], f32r, tag="sq")
                    nc.scalar.activation(out=sq[:], in_=y3[:, dt, cs],
                                         func=AF.Square)
                    sqs[dt] = sq
                    # stats lag one dt so the DVE has a chain-time of slack
                    # before the PE needs its result
                    if dt > 0:
                        nc.tensor.matmul(pst1[0:1, :], ones_r[:, 0:1],
                                         y3[:, dt - 1, cs],
                                         start=(dt == 1), stop=False)
                        nc.tensor.matmul(pst2[0:1, :], ones_r[:, 0:1],
                                         sqs[dt - 1][:],
                                         start=(dt == 1), stop=False)
                nc.tensor.matmul(pst1[0:1, :], ones_r[:, 0:1],
                                 y3[:, NDT - 1, cs], start=False, stop=True)
                nc.tensor.matmul(pst2[0:1, :], ones_r[:, 0:1],
                                 sqs[NDT - 1][:], start=False, stop=True)
                return pst1, pst2

            pst1, pst2 = w2_half(0)
            mk, in_psum = ln_stats(pst1, pst2, HQ, 0)
            ln_norm(mk, in_psum, y3, slice(0, HQ), ln_sb[3][0], ln_sb[3][1],
                    None, out_t)
            pst1, pst2 = w2_half(1)
            mk, in_psum = ln_stats(pst1, pst2, HQ, HQ, pe_bcast=True)
            ln_norm(mk, in_psum, y3, slice(HQ, SQ), ln_sb[3][0], ln_sb[3][1],
                    None, out_t)

    nc.compile()
    return nc


def _qrows(h):
    return np.concatenate(
        [np.arange(64 * (2 * t + h), 64 * (2 * t + h) + 64) for t in range(8)])


def _prepare_in_maps(inputs):
    f = np.float32
    di = np.asarray(inputs["decoder_input"], f)
    eo = np.asarray(inputs["encoder_output"], f)
    mask = np.asarray(inputs["mask"])

    def b16(a):
        return np.ascontiguousarray(a).astype(BF16)

    def wmat(w):  # (H, D, DH) -> (D, H*DH)
        return np.transpose(np.asarray(w, f), (1, 0, 2)).reshape(D, H * DH)

    def colmajor(w, no, co):  # [D_in, N_out] -> [no, 128, D_in/128, co]
        return w.reshape(w.shape[0] // 128, 128, no, co).transpose(2, 1, 0, 3)

    def pmajor(xt, n):  # [D, n] (feature-major) -> [128, NDT, n]
        return np.ascontiguousarray(
            xt.reshape(NDT, 128, n).transpose(1, 0, 2))

    shared = {}
    vecs = {}
    for p in ("sa", "ca"):
        shared[f"w_{p}_q"] = b16(colmajor(wmat(inputs[f"{p}_wq"]), NHP, 128))
        shared[f"w_{p}_k"] = b16(colmajor(wmat(inputs[f"{p}_wk"]), NHP, 128))
        shared[f"w_{p}_v"] = b16(colmajor(wmat(inputs[f"{p}_wv"]), 2, 512))
        wo = np.asarray(inputs[f"{p}_wo"], f)
        shared[f"w_{p}_o"] = b16(colmajor(wo, NDT, 128))
        vecs[f"bq_{p}"] = np.asarray(inputs[f"{p}_bq"], f).reshape(H * DH)
        bv = np.asarray(inputs[f"{p}_bv"], f).reshape(H * DH)
        vecs[f"bo_{p}"] = np.asarray(inputs[f"{p}_bo"], f) + bv @ wo
    shared["w_ff1"] = b16(colmajor(np.asarray(inputs["ff_w1"], f), NFT, 128))
    w2cm = b16(colmajor(np.asarray(inputs["ff_w2"], f), NDT, 128))
    shared["w_ff2"] = w2cm
    # dt pairs (0,1) and (2,3) pre-packed as [2][128, 2, NFT, 128]
    shared["w_ff2p"] = np.ascontiguousarray(
        w2cm[0:4].reshape(2, 2, 128, NFT, 128).transpose(0, 2, 1, 3, 4))

    def cols(v, n):  # [n*128] -> [128, n]
        return np.asarray(v, f).reshape(n, 128).T

    va = np.concatenate([
        cols(vecs["bq_sa"], NHP), cols(vecs["bq_ca"], NHP),
        cols(vecs["bo_sa"], NDT), cols(vecs["bo_ca"], NDT),
        cols(inputs["ff_b2"], NDT),
        cols(inputs["ln1_g"], NDT), cols(inputs["ln1_b"], NDT),
        cols(inputs["ln2_g"], NDT), cols(inputs["ln2_b"], NDT),
        cols(inputs["ln3_g"], NDT), cols(inputs["ln3_b"], NDT),
        cols(inputs["ff_b1"], NFT),
    ], axis=1)
    shared["v_all"] = np.ascontiguousarray(va, dtype=f)

    qr = {h: _qrows(h) for h in (0, 1)}
    in_maps = []
    for c in range(NCORES):
        b, h = divmod(c, 2)
        X = di[b]
        m = dict(shared)
        m["xt_full"] = b16(pmajor(X.T, S))
        Xq = X[qr[h]]
        m["xq"] = b16(pmajor(Xq.T, SQ))
        m["enc_t"] = b16(pmajor(eo[b].T, S))
        mb = mask[b][qr[h]].astype(f)          # [SQ q, S k]
        slabs = np.zeros((NKC, 128, 64), f)
        for j in range(NKC):
            slabs[j] = mb[64 * j:64 * j + 64, 128 * j:128 * (j + 1)].T
        m["sa_mask"] = np.ascontiguousarray(
            slabs.transpose(1, 0, 2)).astype(BF16)
        in_maps.append(m)
    return in_maps


def _collect_output(results):
    qr = {h: _qrows(h) for h in (0, 1)}
    out = np.zeros((B, S, D), np.float32)
    for c in range(NCORES):
        b, h = divmod(c, 2)
        ot = np.asarray(results[c]["out_t"], np.float32).reshape(D, SQ)
        out[b, qr[h]] = ot.T
    return out


def kernel(**inputs):
    global _PROG
    if _PROG is None:
        _PROG = _build_program()
    from concourse.bass_utils import run_bass_kernel_spmd

    in_maps = _prepare_in_maps(inputs)
    res = run_bass_kernel_spmd(_PROG, in_maps, list(range(NCORES)))
    if res.exec_time_ns is not None:
        print(f"HW exec time: {res.exec_time_ns} ns")
    return _collect_output(res.results)
